# revision 5
# baseline (speedup 1.0000x reference)
"""Trainium2 Bass kernel for nn_Attention (dense transformer attention block).

Reference computation (per batch b):
  q = BN(wq @ x) -> (8 heads, 16, 3136);  k likewise;  v = BN(wv @ x) -> (8, 64, 3136)
  attn = softmax(q^T k) over 3136x3136 tokens (no 1/sqrt(d) scaling)
  o = attn @ v^T -> (512, 56, 56);  out = BN(wp @ o) -> (256, 56, 56)

Sharding: 8 cores = 2 batches x 4 query-token chunks of 784. Each core
computes k/v for all 3136 key tokens (cheap, redundant) and attention +
output projection for its own 784 query tokens. Zero collectives.

Key algebraic simplifications (host-side folding):
  - k-bias drops entirely: softmax over keys is invariant to per-query
    logit shifts, and (q+bq)^T bk is constant across keys.
  - v-bias folds into the output-projection bias: softmax rows sum to 1,
    so o_norm = o_raw_norm + bv, and pb_eff = bp + wp_eff^T bv.
  - q/k projections emit DIRECTLY into the zero-padded 32-row-aligned
    per-head layout via host-padded weight columns (no regroup DMAs).

Device algorithm per core (bf16 matmuls, f32 PSUM):
  - scores S_T[key, query] per (head-pair, chunk of 384|400 queries) in
    3-block PSUM groups; per-head 32-aligned row groups + +32-shifted
    replicas so consecutive blocks/heads use distinct PE row groups.
  - exp split across engines: ACT (scalar.activation Exp) for most block
    groups; DVE / GPSIMD handle some groups via a Schraudolph-style
    int16 bit trick that emits bf16 2^t bits directly (tensor_scalar
    mult-add, f32 PSUM in -> int16 SBUF out). The global exp(s)*2^-7
    scale cancels in the softmax divide.
  - o'^T accumulation TRANSPOSED: stationary = probs tile p[k-block,
    128-query tile] (full 128x128 PE usage), moving = v'^T[k, 65] whose
    ones-column accumulates the softmax denominator. N=65-cycle matmuls.
  - softmax divide: per-partition scalar (reciprocal of column 64,
    tensor_scalar multiply) -- no cross-partition broadcast needed.
  - o tiles transposed back to [dims, queries] on the PE (bf16 transpose
    into a carved PSUM slot), evacuated by GPSIMD into of_sb.
  - out = wp_pair @ of (K=128 per head-pair), bias on DVE, DMA out.
  - Software pipeline: iteration i's o'T matmuls execute as PE filler
    inside iteration i+1's scores/exp phase.
"""

import os
import sys

for _p in ("/opt/trn_rl_repo", "/root/.axon_site/_ro/trn_rl_repo"):
    if os.path.isdir(_p) and _p not in sys.path:
        sys.path.insert(0, _p)

import numpy as np

NUM_HEADS = 8
KEY_DIM = 16
D_HEAD = 64
B = 2
C = 256
HH = 56
WW = 56
N = HH * WW          # 3136 tokens
NCHUNK = N // 4      # 784 query tokens per core
NB = (N + 127) // 128            # 25 key-blocks
MB_SIZES = [128] * 24 + [64]
KS = [128, 128]                  # contraction chunks for K=256
CW = [384, 400]                  # query sub-chunk widths per pair
CO = [0, 384]
PAIRS = [(0, 2), (1, 3), (4, 6), (5, 7)]
GROUPS = [list(range(g * 3, min(g * 3 + 3, NB))) for g in range(9)]
# exp engine per score group: ACT carries most, DVE offloads 9 blocks via
# the Schraudolph bit trick (adds ~2e-3 rel err, see module docstring).
# GPSIMD cannot read PSUM, so it only handles DMAs/memsets.
EXP_ENG = ['act', 'act', 'dve', 'act', 'act', 'dve', 'act', 'dve', 'act']
# o'T query tiles per chunk: (offset within chunk, M)
QT = [[(0, 128), (128, 128), (256, 128)],
      [(0, 128), (128, 128), (256, 128), (384, 16)]]
EA = 184.66496523378732   # 2^7 * log2(e)
EB = 15352.75             # 127*2^7 - 7*2^7 + sigma (sigma ~ -7.25)
LN2_7 = -4.852030263919617  # -7*ln2 exp bias; cancels in the divide

_GRAPH = None


def _build_graph():
    import concourse.bass as bass  # noqa: F401
    import concourse.mybir as mybir
    import concourse.tile as tile
    from concourse import bacc
    from contextlib import ExitStack

    f32 = mybir.dt.float32
    bf16 = mybir.dt.bfloat16
    i16 = mybir.dt.int16
    Exp = mybir.ActivationFunctionType.Exp
    MUL = mybir.AluOpType.mult
    ADD = mybir.AluOpType.add

    nc = bacc.Bacc("TRN2", target_bir_lowering=False, debug=False, num_devices=8)
    xa_d = nc.dram_tensor("xa", [256, N], bf16, kind="ExternalInput").ap()
    xq_d = nc.dram_tensor("xq", [256, NCHUNK], bf16, kind="ExternalInput").ap()
    # wqk layout: [256, 4, 128], L index: 0=q_lo 1=q_hi 2=k_lo 3=k_hi
    wqk_d = nc.dram_tensor("wqk", [256, 4, 128], bf16, kind="ExternalInput").ap()
    wv_d = nc.dram_tensor("wv", [256, 520], bf16, kind="ExternalInput").ap()
    wp_d = nc.dram_tensor("wp", [128, 4, 256], bf16, kind="ExternalInput").ap()
    qb_d = nc.dram_tensor("qb", [128, 2], f32, kind="ExternalInput").ap()
    pb_d = nc.dram_tensor("pb", [128, 2], f32, kind="ExternalInput").ap()
    id_d = nc.dram_tensor("ident", [128, 128], bf16, kind="ExternalInput").ap()
    out_d = nc.dram_tensor("out", [256, NCHUNK], f32, kind="ExternalOutput").ap()

    with tile.TileContext(nc) as tc, ExitStack() as stk:
        const = stk.enter_context(tc.tile_pool(name="const", bufs=1))
        xa_sb = const.tile([128, 2, N], bf16, tag="xa")
        xq_sb = const.tile([128, 2, NCHUNK], bf16, tag="xq")
        wqk_sb = const.tile([128, 2, 4, 128], bf16, tag="wqk")
        wv_sb = const.tile([128, 2, 520], bf16, tag="wv")
        wp_sb = const.tile([128, 4, 256], bf16, tag="wp")
        qb_sb = const.tile([128, 2], f32, tag="qb")
        pb_sb = const.tile([128, 2], f32, tag="pb")
        eb_sb = const.tile([128, 1], f32, tag="eb")
        id_sb = const.tile([128, 128], bf16, tag="ident")
        # per-head padded layouts: head h -> (lo if h<4 else hi) rows
        # [32*(h%4), +16), zeros elsewhere; replica r=1 shifted +32 rows
        k_ly = [[const.tile([128, N], bf16, tag=f"k{L}{r}", name=f"k{L}{r}")
                 for r in range(2)] for L in range(2)]
        q_ly = [[const.tile([128, NCHUNK], bf16, tag=f"q{L}{r}", name=f"q{L}{r}")
                 for r in range(2)] for L in range(2)]
        # v'^T: [token-in-block, block, head-half, 65*h' + (64 dims + ones)]
        vT_sb = const.tile([128, NB, 2, 264], bf16, tag="vt")
        of_sb = const.tile([128, 4, NCHUNK], bf16, tag="of")
        y_sb = const.tile([128, 2, NCHUNK], f32, tag="y")

        for kc in range(2):
            off = 128 * kc
            nc.sync.dma_start(out=xa_sb[:, kc, :], in_=xa_d[off:off + 128, :])
            nc.sync.dma_start(out=xq_sb[:, kc, :], in_=xq_d[off:off + 128, :])
            nc.sync.dma_start(out=wqk_sb[:, kc, :, :],
                              in_=wqk_d[off:off + 128, :, :])
            nc.sync.dma_start(out=wv_sb[:, kc, :], in_=wv_d[off:off + 128, :])
        nc.sync.dma_start(out=wp_sb[:], in_=wp_d[:])
        nc.sync.dma_start(out=qb_sb[:], in_=qb_d)
        nc.sync.dma_start(out=pb_sb[:], in_=pb_d)
        nc.sync.dma_start(out=id_sb[:], in_=id_d)
        nc.vector.memset(eb_sb[:], LN2_7)

        # ---- projections ----
        with tc.tile_pool(name="psA", bufs=2, space="PSUM") as psA, \
             tc.tile_pool(name="psAV", bufs=3, space="PSUM") as psAV:
            # q projection, directly in padded layout (bias on ACT evac)
            for L in range(2):
                for c2 in range(2):
                    q_ps = psA.tile([128, 512], f32, tag="qk", name="qps")
                    for kc in range(2):
                        nc.tensor.matmul(
                            q_ps[0:128, 0:392],
                            wqk_sb[0:KS[kc], kc, L, :],
                            xq_sb[0:KS[kc], kc, c2 * 392:(c2 + 1) * 392],
                            start=(kc == 0), stop=(kc == 1))
                    nc.scalar.add(
                        q_ly[L][0][:, c2 * 392:(c2 + 1) * 392],
                        q_ps[0:128, 0:392], qb_sb[:, L:L + 1])
            # k projection (no bias -- cancels in softmax)
            for L in range(2):
                for p7 in range(7):
                    c0 = 448 * p7
                    k_ps = psA.tile([128, 512], f32, tag="qk", name="kps")
                    for kc in range(2):
                        nc.tensor.matmul(
                            k_ps[0:128, 0:448],
                            wqk_sb[0:KS[kc], kc, 2 + L, :],
                            xa_sb[0:KS[kc], kc, c0:c0 + 448],
                            start=(kc == 0), stop=(kc == 1))
                    if L == 0:
                        nc.scalar.copy(k_ly[0][0][:, c0:c0 + 448],
                                       k_ps[0:128, 0:448])
                    else:
                        nc.vector.tensor_copy(k_ly[1][0][:, c0:c0 + 448],
                                              k_ps[0:128, 0:448])
            # +32-row-shifted replicas (distinct PE row groups for
            # consecutive key blocks)
            for pairt in (k_ly[0], k_ly[1], q_ly[0], q_ly[1]):
                src, dst = pairt[0], pairt[1]
                nc.gpsimd.dma_start(out=dst[32:128, :], in_=src[0:96, :])
                nc.gpsimd.dma_start(out=dst[0:32, :], in_=src[96:128, :])
            # v'^T projection
            for mb in range(NB):
                pb_ = MB_SIZES[mb]
                vt_ps = psAV.tile([128, 2, 512], f32, tag="vtps", name="vtps")
                for half in range(2):
                    for kc in range(2):
                        nc.tensor.matmul(
                            vt_ps[0:pb_, half, 0:260],
                            xa_sb[0:KS[kc], kc, mb * 128:mb * 128 + pb_],
                            wv_sb[0:KS[kc], kc, half * 260:(half + 1) * 260],
                            start=(kc == 0), stop=(kc == 1))
                if mb % 2 == 0:
                    nc.scalar.copy(vT_sb[0:pb_, mb, :, 0:260],
                                   vt_ps[0:pb_, :, 0:260])
                else:
                    nc.vector.tensor_copy(vT_sb[0:pb_, mb, :, 0:260],
                                          vt_ps[0:pb_, :, 0:260])
            # denominator ones-columns (after evacs; subtile deps order it)
            for hh in range(2):
                for j in range(4):
                    c1 = 65 * j + 64
                    nc.gpsimd.memset(vT_sb[:, :, hh, c1:c1 + 1], 1.0)

        # ---- main attention loop ----
        ITERS = [(pi, c) for c in range(2) for pi in range(4)]

        with tc.tile_pool(name="pP", bufs=4) as pP, \
             tc.tile_pool(name="pEp", bufs=6) as pEp, \
             tc.tile_pool(name="pOs", bufs=3) as pOs, \
             tc.tile_pool(name="psS", bufs=2, space="PSUM") as psS, \
             tc.tile_pool(name="psO", bufs=2, space="PSUM") as psO:

            def emit_wp(c):
                W, co = CW[c], CO[c]
                for mo in range(2):
                    y_ps = psO.tile([128, 512], f32, tag="ot", name=f"y{mo}")
                    for j in range(4):
                        nc.tensor.matmul(
                            y_ps[0:128, 0:W],
                            wp_sb[0:128, j, mo * 128:(mo + 1) * 128],
                            of_sb[0:128, j, co:co + W],
                            start=(j == 0), stop=(j == 3))
                    nc.vector.tensor_scalar_add(
                        y_sb[:, mo, co:co + W], y_ps[0:128, 0:W],
                        pb_sb[:, mo:mo + 1])
                    nc.sync.dma_start(out=out_d[mo * 128:(mo + 1) * 128,
                                                co:co + W],
                                      in_=y_sb[:, mo, co:co + W])

            def make_fillers(ppi, pc, p_t):
                ppair = PAIRS[ppi]
                W, co = CW[pc], CO[pc]
                jobs = []
                state = {}

                def alloc_ot(t, e):
                    def run():
                        state[('ot', t, e)] = psO.tile(
                            [128, 512], f32, tag="ot", name=f"ot{t}{e}")
                    return run

                def mm(t, e, mb):
                    qo, M = QT[pc][t]
                    def run():
                        ot = state[('ot', t, e)]
                        h = ppair[e]
                        pbi = MB_SIZES[mb]
                        nc.tensor.matmul(
                            ot[0:M, 0:65],
                            p_t[e][0:pbi, mb, qo:qo + M],
                            vT_sb[0:pbi, mb, h // 4,
                                  65 * (h % 4):65 * (h % 4) + 65],
                            start=(mb == 0), stop=(mb == NB - 1))
                    return run

                def div(t, e):
                    qo, M = QT[pc][t]
                    def run():
                        ot = state[('ot', t, e)]
                        if e == 0:
                            state[('os', t)] = pOs.tile(
                                [128, 2, 64], bf16, tag="os", name=f"os{t}")
                        os_t = state[('os', t)]
                        rec = pEp.tile([128, 1], f32, tag="rec", name="rec")
                        nc.vector.reciprocal(rec[0:M, :], ot[0:M, 64:65])
                        nc.vector.tensor_scalar_mul(
                            os_t[0:M, e, :], ot[0:M, 0:64], rec[0:M, 0:1])
                    return run

                def tp(t):
                    qo, M = QT[pc][t]
                    def run():
                        os_t = state[('os', t)]
                        otb = state[('ot', t, 1)].bitcast(bf16)
                        nc.tensor.transpose(
                            otb[0:128, 512:512 + M],
                            os_t[0:M, :, :],
                            id_sb[0:M, 0:M])
                        if t % 2 == 0:
                            nc.scalar.copy(
                                of_sb[0:128, ppi, co + qo:co + qo + M],
                                otb[0:128, 512:512 + M])
                        else:
                            nc.vector.tensor_copy(
                                of_sb[0:128, ppi, co + qo:co + qo + M],
                                otb[0:128, 512:512 + M])
                    return run

                for t in range(len(QT[pc])):
                    for e in range(2):
                        jobs.append(alloc_ot(t, e))
                        for mb in range(NB):
                            jobs.append(mm(t, e, mb))
                        jobs.append(div(t, e))
                    jobs.append(tp(t))
                return jobs

            prev = None
            for it in range(len(ITERS) + 1):
                cur = ITERS[it] if it < len(ITERS) else None
                fillers = []
                if prev is not None:
                    fillers = make_fillers(*prev)
                if cur is None:
                    for job in fillers:
                        job()
                    emit_wp(1)
                    break
                pi, c = cur
                pair = PAIRS[pi]
                W, co = CW[c], CO[c]
                p_t = [pP.tile([128, NB, 400], bf16, tag="p", name=f"p{e}")
                       for e in range(2)]
                nf = len(fillers)
                for g, blocks in enumerate(GROUPS):
                    s_ps = [psS.tile([128, 3, 512], f32, tag="sps",
                                     name=f"sps{e}") for e in range(2)]
                    for i, mb in enumerate(blocks):
                        pbi = MB_SIZES[mb]
                        r = mb % 2
                        for e, h in enumerate(pair):
                            kt = k_ly[h // 4][r]
                            qt = q_ly[h // 4][r]
                            base = (32 * (h % 4) + 32 * r) % 128
                            nc.tensor.matmul(
                                s_ps[e][0:pbi, i, 0:W],
                                kt[base:base + 32, mb * 128:mb * 128 + pbi],
                                qt[base:base + 32, co:co + W],
                                start=True, stop=True,
                                tile_position=(base, 0))
                    gl = len(blocks)
                    gpb = MB_SIZES[blocks[-1]]
                    g0 = blocks[0]
                    for e in range(2):
                        if EXP_ENG[g] == 'act':
                            nc.scalar.activation(
                                out=p_t[e][0:gpb, g0:g0 + gl, 0:W],
                                in_=s_ps[e][0:gpb, 0:gl, 0:W], func=Exp,
                                bias=eb_sb[0:gpb, 0:1])
                        else:
                            eng = nc.vector if EXP_ENG[g] == 'dve' \
                                else nc.gpsimd
                            eng.tensor_scalar(
                                out=p_t[e][0:gpb, g0:g0 + gl, 0:W]
                                .bitcast(i16),
                                in0=s_ps[e][0:gpb, 0:gl, 0:W],
                                scalar1=EA, scalar2=EB, op0=MUL, op1=ADD)
                    lo = nf * max(0, g - 1) // 8
                    hi = nf * g // 8
                    for job in fillers[lo:hi]:
                        job()
                if prev is not None and prev[0] == 3 and prev[1] == 0:
                    emit_wp(0)
                prev = (pi, c, p_t)

    nc.compile()
    return nc


def get_graph():
    global _GRAPH
    if _GRAPH is None:
        _GRAPH = _build_graph()
    return _GRAPH


def make_in_maps(x, wq, sq, bq, wk, sk, bk, wv, sv, bv, wp, sp, bp):
    import ml_dtypes
    bf = ml_dtypes.bfloat16
    f = np.float32
    x2 = np.asarray(x, f).reshape(B, C, N)
    wq = np.asarray(wq, f); sq = np.asarray(sq, f); bq = np.asarray(bq, f)
    wk = np.asarray(wk, f); sk = np.asarray(sk, f)
    wv = np.asarray(wv, f); sv = np.asarray(sv, f); bv = np.asarray(bv, f)
    wp = np.asarray(wp, f); sp = np.asarray(sp, f); bp = np.asarray(bp, f)

    wq_eff = (wq * sq[:, None]).T.astype(f)           # (256, 128)
    wk_eff = (wk * sk[:, None]).T.astype(f)           # k-bias dropped

    # padded per-head layouts: lo = heads 0-3, hi = heads 4-7;
    # head g of a half -> columns [32g, 32g+16), zeros elsewhere
    wqk_arr = np.zeros((256, 4, 128), f)
    qb_arr = np.zeros((128, 2), f)
    for g in range(4):
        wqk_arr[:, 0, 32 * g:32 * g + 16] = wq_eff[:, 16 * g:16 * g + 16]
        wqk_arr[:, 1, 32 * g:32 * g + 16] = wq_eff[:, 16 * (g + 4):16 * (g + 4) + 16]
        wqk_arr[:, 2, 32 * g:32 * g + 16] = wk_eff[:, 16 * g:16 * g + 16]
        wqk_arr[:, 3, 32 * g:32 * g + 16] = wk_eff[:, 16 * (g + 4):16 * (g + 4) + 16]
        qb_arr[32 * g:32 * g + 16, 0] = bq[16 * g:16 * g + 16]
        qb_arr[32 * g:32 * g + 16, 1] = bq[16 * (g + 4):16 * (g + 4) + 16]

    wv_base = wv * sv[:, None]  # (512, 256); v-bias folded into pb_eff
    wv_arr = np.zeros((256, 520), f)
    for h in range(NUM_HEADS):
        col = 260 * (h // 4) + 65 * (h % 4)
        wv_arr[:, col:col + 64] = wv_base[64 * h:64 * h + 64, :].T

    wp_eff = (wp * sp[:, None]).T.astype(f)  # (512, 256), row c = 64h+d
    wp_arr = np.zeros((128, 4, 256), f)
    for p, (hA, hB) in enumerate(PAIRS):
        wp_arr[0:64, p, :] = wp_eff[64 * hA:64 * hA + 64, :]
        wp_arr[64:128, p, :] = wp_eff[64 * hB:64 * hB + 64, :]
    pb_eff = bp + wp_eff.T @ bv              # fold v-bias through wp
    pb_arr = pb_eff.reshape(2, 128).T.copy()  # (128, 2)

    ident = np.eye(128, dtype=f)
    in_maps = []
    for core in range(8):
        b, j = core // 4, core % 4
        xa_full = np.ascontiguousarray(x2[b])
        xq_c = np.ascontiguousarray(xa_full[:, j * NCHUNK:(j + 1) * NCHUNK])
        in_maps.append(dict(
            xa=xa_full.astype(bf), xq=xq_c.astype(bf),
            wqk=wqk_arr.astype(bf), wv=wv_arr.astype(bf),
            wp=wp_arr.astype(bf),
            qb=qb_arr.astype(f), pb=pb_arr.astype(f),
            ident=ident.astype(bf)))
    return in_maps


def assemble_output(results):
    y = np.zeros((B, C, N), np.float32)
    for core in range(8):
        b, j = core // 4, core % 4
        y[b, :, j * NCHUNK:(j + 1) * NCHUNK] = results[core]["out"]
    return y.reshape(B, C, HH, WW)


def kernel(**inputs):
    from concourse.bass_utils import run_bass_kernel_spmd
    nc = get_graph()
    in_maps = make_in_maps(**inputs)
    res = run_bass_kernel_spmd(nc, in_maps, core_ids=list(range(8)))
    return assemble_output(res.results)


if __name__ == "__main__":
    rng = np.random.default_rng(0)
    ins = dict(
        x=rng.standard_normal((2, 256, 56, 56), np.float32),
        wq=rng.standard_normal((128, 256), np.float32) * 0.05,
        sq=rng.random(128, np.float32),
        bq=rng.standard_normal(128, np.float32) * 0.05,
        wk=rng.standard_normal((128, 256), np.float32) * 0.05,
        sk=rng.random(128, np.float32),
        bk=rng.standard_normal(128, np.float32) * 0.05,
        wv=rng.standard_normal((512, 256), np.float32) * 0.05,
        sv=rng.random(512, np.float32),
        bv=rng.standard_normal(512, np.float32) * 0.05,
        wp=rng.standard_normal((256, 512), np.float32) * 0.05,
        sp=rng.random(256, np.float32),
        bp=rng.standard_normal(256, np.float32) * 0.05,
    )
    out = kernel(**ins)
    print("out", out.shape, out.dtype, float(np.abs(out).mean()))
